# revision 7
# baseline (speedup 1.0000x reference)
"""Trainium2 Bass kernel for nn_BLLoss_66494683676972.

Contrastive (SimCLR-like) loss over rep = [normalize(emb_i); normalize(emb_j)]
(n=8192 rows, D=512):

    sim = rep @ rep.T
    nom = sum(exp(2*diag(sim, +-{B, 2B, 3B})))          (B=2048)
    den = sum_{i!=j} exp(2*sim) - nom
    loss = -log(nom/den) / 8192

Sharding: sim is symmetric, so only a cyclic half-band is computed.  Rows are
split into 16 chunks of 512; chunk R needs column-chunks R+1..R+7 (and R+8 for
R<=7) plus its diagonal block.  Core c owns chunks {c, 15-c} -> 17 blocks of
512x512 per core, perfectly balanced.  Per-core column data is rotated on the
host so the SPMD device program uses only static offsets.

The positive diagonals and the main diagonal are extracted from the computed
blocks with mask-reduce ops (t=4 blocks carry d=+2048 / d=+6144-mirror pairs,
t=8 blocks carry d=+4096 pairs).  Each core emits 4 partial sums; the host
combines them (the gather/unshard step) into the scalar loss.
"""

import numpy as np

import concourse.bass as bass
import concourse.tile as tile
from concourse import bacc, mybir
from concourse.bass_utils import run_bass_kernel_spmd

B = 2048
N = 4 * B            # 8192 rows in rep
D = 512
NCORES = 8
CHUNK = 512          # row-chunk granularity (16 chunks)
NCHUNKS = N // CHUNK
TAU = 0.5
SCALE = 1.0 / TAU    # 2.0

ROWS_LOC = 2 * CHUNK          # 1024
ROWS_A = 8 * CHUNK            # 4096   col chunks +1..+8 of chunkA
ROWS_B = 7 * CHUNK            # 3584   col chunks +1..+7 of chunkB
ROWS_ALL = ROWS_LOC + ROWS_A + ROWS_B  # 8704

F32 = mybir.dt.float32
BF16 = mybir.dt.bfloat16

_CACHED = {}


def _build_program():
    """Build (nc, out_name) for the SPMD program run on each of the 8 cores.

    KBUILD_STAGE env (debug bisection): 1=norm only, 2=+transpose loads,
    3=+diag blocks, 4=+jobA, 5=full (default).
    """
    import os
    stage = int(os.environ.get("KBUILD_STAGE", "5"))
    nc = bacc.Bacc("TRN2", target_bir_lowering=False, debug=False)

    loc_d = nc.declare_dram_parameter("loc", [ROWS_LOC, D], F32, isOutput=False)
    cols_d = nc.declare_dram_parameter("cols", [ROWS_A + ROWS_B, D], F32, isOutput=False)
    masks_d = nc.declare_dram_parameter("masks", [4, 128, D], F32, isOutput=False)
    out_d = nc.declare_dram_parameter("out", [1, 4], F32, isOutput=True)

    # bf16 normalized-row scratch, one region per source so the transposed
    # reloads only wait on their own region's stores.
    zloc_d = nc.dram_tensor("zloc_scratch", [ROWS_LOC, D], BF16)
    za_d = nc.dram_tensor("za_scratch", [ROWS_A, D], BF16)
    zb_d = nc.dram_tensor("zb_scratch", [ROWS_B, D], BF16)

    with tile.TileContext(nc) as tc:
        with (
            tc.tile_pool(name="persist", bufs=1) as persist,
            tc.tile_pool(name="xin", bufs=4) as xin_pool,
            tc.tile_pool(name="zrow", bufs=4) as zrow_pool,
            tc.tile_pool(name="vecs", bufs=8) as vec_pool,
            tc.tile_pool(name="scratch", bufs=2) as scr_pool,
            tc.tile_pool(name="expout", bufs=4) as exp_pool,
            tc.tile_pool(name="psum", bufs=8, space=bass.MemorySpace.PSUM) as psum_pool,
        ):
            # ---- persistent SBUF tensors ----
            masks = persist.tile([128, 4, D], F32)
            nc.gpsimd.dma_start(out=masks, in_=masks_d.ap().rearrange("s p c -> p s c"))

            # zT layout: [128 partitions (feature-within-k-chunk), k-chunk, cols]
            zlocT = persist.tile([128, 4, ROWS_LOC], BF16)
            zTA = persist.tile([128, 4, ROWS_A], BF16)
            zTB = persist.tile([128, 4, ROWS_B], BF16)

            ones = persist.tile([128, 1], F32)
            nc.vector.memset(ones, 1.0)

            # accumulator columns: one fp32 scalar per [128,512] tile processed
            NT_OFF = 60   # 32 jobA + 28 jobB off-diag block tiles
            NT_Q = 8      # diagA + diagB block tiles
            NT_D = 8      # main-diag extractions (from diag blocks)
            NT_NP = 12    # positive extractions (t4A, t8A, t4B)
            acc_off = persist.tile([128, NT_OFF], F32)
            acc_q = persist.tile([128, NT_Q], F32)
            acc_d = persist.tile([128, NT_D], F32)
            acc_np = persist.tile([128, NT_NP], F32)
            for a in (acc_off, acc_q, acc_d, acc_np):
                nc.vector.memset(a, 0.0)

            # ---- phase 1: load rows, normalize, cast bf16, store to scratch ----
            def norm_region(src_ap, dst_dram, nrows):
                ntiles = nrows // 128
                for t in range(ntiles):
                    xin = xin_pool.tile([128, D], F32)
                    nc.gpsimd.dma_start(out=xin, in_=src_ap[t * 128:(t + 1) * 128, :])
                    scr = scr_pool.tile([128, D], F32, tag="normscr")
                    nc.vector.tensor_mul(out=scr, in0=xin, in1=xin)
                    sq = vec_pool.tile([128, 1], F32, tag="sq")
                    nc.vector.reduce_sum(out=sq, in_=scr,
                                         axis=mybir.AxisListType.X)
                    rn = vec_pool.tile([128, 1], F32, tag="rn")
                    nc.scalar.activation(
                        out=rn, in_=sq,
                        func=mybir.ActivationFunctionType.Abs_reciprocal_sqrt)
                    zrow = zrow_pool.tile([128, D], BF16)
                    nc.vector.tensor_scalar_mul(out=zrow, in0=xin, scalar1=rn)
                    nc.scalar.dma_start(
                        out=dst_dram[t * 128:(t + 1) * 128, :], in_=zrow)

            # ---- transposed reloads: [512 rows, 128 feat] -> [128, 512] ----
            def load_zT(dst, src_dram, nrows):
                for k in range(4):
                    for j in range(nrows // CHUNK):
                        nc.sync.dma_start_transpose(
                            out=dst[:, k, j * CHUNK:(j + 1) * CHUNK],
                            in_=src_dram[j * CHUNK:(j + 1) * CHUNK,
                                         k * 128:(k + 1) * 128],
                        )

            # ---- matmul block: lhsT cols [m0..m0+512) of zlocT vs 512 rhs cols ----
            def do_block(lhs_m0, rhs, rhs_n0, acc, acc_idx, extract, eacc=None, eidx=0):
                """One 512x512 sim block: 4 m-tiles x (4 k accum) matmuls + exp."""
                for m in range(4):
                    ps = psum_pool.tile([128, CHUNK], F32, tag="mm")
                    for k in range(4):
                        nc.tensor.matmul(
                            ps,
                            zlocT[:, k, lhs_m0 + m * 128: lhs_m0 + (m + 1) * 128],
                            rhs[:, k, rhs_n0: rhs_n0 + CHUNK],
                            start=(k == 0), stop=(k == 3),
                        )
                    if extract:
                        ex = exp_pool.tile([128, CHUNK], F32, tag="exp")
                    else:
                        ex = scr_pool.tile([128, CHUNK], F32, tag="expscr")
                    nc.scalar.activation(
                        out=ex, in_=ps, func=mybir.ActivationFunctionType.Exp,
                        scale=SCALE, accum_out=acc[:, acc_idx + m: acc_idx + m + 1],
                    )
                    if extract:
                        scr = scr_pool.tile([128, CHUNK], F32, tag="extscr")
                        nc.vector.tensor_mul(out=scr, in0=ex, in1=masks[:, m, :])
                        nc.vector.reduce_sum(
                            out=eacc[:, eidx + m: eidx + m + 1], in_=scr,
                            axis=mybir.AxisListType.X)

            # ---------------- emission order (pipelining-friendly) ----------
            norm_region(loc_d.ap(), zloc_d.ap(), ROWS_LOC)
            if stage >= 2:
                load_zT(zlocT, zloc_d.ap(), ROWS_LOC)

            if stage >= 3:
                # diag blocks first: only depend on zlocT -> PE starts early
                do_block(0, zlocT, 0, acc_q, 0, True, acc_d, 0)       # diagA
                do_block(512, zlocT, 512, acc_q, 4, True, acc_d, 4)   # diagB

            norm_region(cols_d.ap()[:ROWS_A, :], za_d.ap(), ROWS_A)
            if stage >= 2:
                load_zT(zTA, za_d.ap(), ROWS_A)

            if stage >= 4:
                # jobA: chunkA x col-chunks t=1..8 (n=3 -> t4, n=7 -> t8 pos)
                for n in range(8):
                    extract = n in (3, 7)
                    eidx = 0 if n == 3 else 4
                    do_block(0, zTA, n * CHUNK, acc_off, n * 4, extract,
                             acc_np, eidx)

            norm_region(cols_d.ap()[ROWS_A:, :], zb_d.ap(), ROWS_B)
            if stage >= 2:
                load_zT(zTB, zb_d.ap(), ROWS_B)

            n_jobb = 7 if stage >= 5 else int(os.environ.get("KBUILD_JOBB", "0"))
            if n_jobb:
                # jobB: chunkB x col-chunks t=1..7 (n=3 -> t4 pos)
                for n in range(n_jobb):
                    extract = n == 3
                    do_block(512, zTB, n * CHUNK, acc_off, 32 + n * 4, extract,
                             acc_np, 8)

            # ---- final reduction: 4 categories -> [128,1] -> partition sum ----
            fin = persist.tile([128, 4], F32)
            for i, (acc, w) in enumerate(
                    [(acc_off, NT_OFF), (acc_q, NT_Q), (acc_d, NT_D), (acc_np, NT_NP)]):
                nc.vector.reduce_sum(out=fin[:, i:i + 1], in_=acc[:, :w],
                                     axis=mybir.AxisListType.X)
            psf = psum_pool.tile([128, CHUNK], F32, tag="mm")
            nc.tensor.matmul(psf[0:1, 0:4], ones, fin, start=True, stop=True)
            fout = persist.tile([1, 4], F32)
            nc.vector.tensor_copy(out=fout, in_=psf[0:1, 0:4])
            nc.gpsimd.dma_start(out=out_d.ap(), in_=fout)

    nc.compile()
    return nc, "out"


def _host_inputs(emb_i: np.ndarray, emb_j: np.ndarray):
    """Pure slicing/concat: build the 8 per-core input maps."""
    rows = np.ascontiguousarray(
        np.concatenate([emb_i, emb_j], axis=0), dtype=np.float32)

    masks = np.zeros((4, 128, D), dtype=np.float32)
    for s in range(4):
        for p in range(128):
            masks[s, p, 128 * s + p] = 1.0

    def cyc(start_row, nrows):
        idx = (np.arange(start_row, start_row + nrows)) % N
        return rows[idx]

    in_maps = []
    for c in range(NCORES):
        chunk_a, chunk_b = c, 15 - c
        loc = np.concatenate(
            [rows[chunk_a * CHUNK:(chunk_a + 1) * CHUNK],
             rows[chunk_b * CHUNK:(chunk_b + 1) * CHUNK]], axis=0)
        cols_a = cyc((chunk_a + 1) * CHUNK, ROWS_A)
        cols_b = cyc((chunk_b + 1) * CHUNK % N, ROWS_B)
        in_maps.append({
            "loc": np.ascontiguousarray(loc),
            "cols": np.ascontiguousarray(np.concatenate([cols_a, cols_b], axis=0)),
            "masks": masks,
        })
    return in_maps


def _combine(parts):
    """parts: list of 8 arrays [1,4] (S_off, Q, D, Np) -> scalar loss."""
    tot = np.sum(np.stack([p.astype(np.float64).ravel() for p in parts]), axis=0)
    s_off, q, d, npos = tot
    nom = 2.0 * npos
    den = 2.0 * s_off + q - d - nom
    loss = -np.log(nom / den) / N
    return np.float32(loss)


def kernel(emb_i: np.ndarray, emb_j: np.ndarray) -> np.ndarray:
    if "prog" not in _CACHED:
        _CACHED["prog"] = _build_program()
    nc, out_name = _CACHED["prog"]
    in_maps = _host_inputs(np.asarray(emb_i), np.asarray(emb_j))
    res = run_bass_kernel_spmd(nc, in_maps, list(range(NCORES)))
    parts = [res.results[c][out_name] for c in range(NCORES)]
    return np.array(_combine(parts), dtype=np.float32)


# revision 9
# speedup vs baseline: 1.5258x; 1.5258x over previous
"""Trainium2 Bass kernel for nn_BLLoss_66494683676972.

Contrastive (SimCLR-like) loss over rep = [normalize(emb_i); normalize(emb_j)]
(n=8192 rows, D=512):

    sim = rep @ rep.T
    nom = sum(exp(2*diag(sim, +-{B, 2B, 3B})))          (B=2048)
    den = sum_{i!=j} exp(2*sim) - nom
    loss = -log(nom/den) / 8192

Sharding: sim is symmetric, so only a cyclic half-band is computed.  Rows are
split into 16 chunks of 512; chunk R needs column-chunks R+1..R+7 (and R+8 for
R<=7) plus its diagonal block.  Core c owns chunks {c, 15-c} -> 17 blocks of
512x512 per core, perfectly balanced.  Per-core column data is rotated on the
host so the SPMD device program uses only static offsets.

The positive diagonals and the main diagonal are extracted from the computed
blocks with mask-reduce ops (t=4 blocks carry d=+2048 / d=+6144-mirror pairs,
t=8 blocks carry d=+4096 pairs).  Each core emits 4 partial sums; the host
combines them (the gather/unshard step) into the scalar loss.

Pipeline per core: cast-to-bf16 DMA loads -> batched square/reduce (DVE) ->
per-region rsqrt (one ACT table load) -> row scale (DVE) -> bf16 scratch in
DRAM -> xbar DMA-transpose reloads -> bf16 matmuls (PE, fp32 PSUM) -> fused
exp+row-sum (ACT) -> mask-extract diagonals (DVE) -> partition-sum (PE).
"""

import numpy as np

import concourse.bass as bass
import concourse.tile as tile
from concourse import bacc, mybir
from concourse.bass_utils import run_bass_kernel_spmd

B = 2048
N = 4 * B            # 8192 rows in rep
D = 512
NCORES = 8
CHUNK = 512          # row-chunk granularity (16 chunks)
TAU = 0.5
SCALE = 1.0 / TAU    # 2.0

ROWS_LOC = 2 * CHUNK          # 1024
ROWS_A = 8 * CHUNK            # 4096   col chunks +1..+8 of chunkA
ROWS_B = 7 * CHUNK            # 3584   col chunks +1..+7 of chunkB

F32 = mybir.dt.float32
BF16 = mybir.dt.bfloat16

_CACHED = {}


def _build_program():
    """Build (nc, out_name) for the SPMD program run on each of the 8 cores."""
    nc = bacc.Bacc("TRN2", target_bir_lowering=False, debug=False)

    loc_d = nc.declare_dram_parameter("loc", [ROWS_LOC, D], F32, isOutput=False)
    cols_d = nc.declare_dram_parameter("cols", [ROWS_A + ROWS_B, D], F32, isOutput=False)
    masks_d = nc.declare_dram_parameter("masks", [4, 128, D], F32, isOutput=False)
    out_d = nc.declare_dram_parameter("out", [1, 4], F32, isOutput=True)

    # bf16 normalized-row scratch, one region per source so the transposed
    # reloads only wait on their own region's stores.
    zloc_d = nc.dram_tensor("zloc_scratch", [ROWS_LOC, D], BF16)
    za_d = nc.dram_tensor("za_scratch", [ROWS_A, D], BF16)
    zb_d = nc.dram_tensor("zb_scratch", [ROWS_B, D], BF16)

    with tile.TileContext(nc) as tc:
        with (
            tc.tile_pool(name="persist", bufs=1) as persist,
            tc.tile_pool(name="xin", bufs=10) as xin_pool,
            tc.tile_pool(name="zrow", bufs=8) as zrow_pool,
            tc.tile_pool(name="scratch", bufs=2) as scr_pool,
            tc.tile_pool(name="expout", bufs=4) as exp_pool,
            tc.tile_pool(name="psum", bufs=8, space=bass.MemorySpace.PSUM) as psum_pool,
        ):
            # ---- persistent SBUF tensors ----
            masks = persist.tile([128, 4, D], F32)
            nc.gpsimd.dma_start(out=masks, in_=masks_d.ap().rearrange("s p c -> p s c"))

            # zT layout: [128 partitions (feature-within-k-chunk), k-chunk, cols]
            zlocT = persist.tile([128, 4, ROWS_LOC], BF16)
            zTA = persist.tile([128, 4, ROWS_A], BF16)
            zTB = persist.tile([128, 4, ROWS_B], BF16)

            ones = persist.tile([128, 1], F32)
            nc.vector.memset(ones, 1.0)

            # per-region norm vectors (sq sums -> rnorm), one column per row-tile
            sq_loc = persist.tile([128, ROWS_LOC // 128], F32)
            sq_a = persist.tile([128, ROWS_A // 128], F32)
            sq_b = persist.tile([128, ROWS_B // 128], F32)
            rn_loc = persist.tile([128, ROWS_LOC // 128], F32)
            rn_a = persist.tile([128, ROWS_A // 128], F32)
            rn_b = persist.tile([128, ROWS_B // 128], F32)

            # accumulator columns: one fp32 scalar per [128,512] tile processed
            NT_OFF = 60   # 32 jobA + 28 jobB off-diag block tiles
            NT_Q = 8      # diagA + diagB block tiles
            NT_D = 8      # main-diag extractions (from diag blocks)
            NT_NP = 12    # positive extractions (t4A, t8A, t4B)
            acc_off = persist.tile([128, NT_OFF], F32)
            acc_q = persist.tile([128, NT_Q], F32)
            acc_d = persist.tile([128, NT_D], F32)
            acc_np = persist.tile([128, NT_NP], F32)

            # ---- phase 1: cast-load rows (4 tiles/load), square+reduce ----
            def norm_region(src_ap, dst_dram, nrows, sq, rn):
                ntiles = nrows // 128
                xbs = []
                for g in range(ntiles // 4):
                    xb = xin_pool.tile([128, 4, D], BF16)
                    # bf16 cast during SWDGE DMA; rows 512g..512g+512
                    nc.gpsimd.dma_start(
                        out=xb,
                        in_=src_ap[512 * g: 512 * (g + 1), :].rearrange(
                            "(a p) d -> p a d", p=128))
                    scr = scr_pool.tile([128, 4, D], BF16, tag="normscr")
                    nc.vector.tensor_mul(out=scr, in0=xb, in1=xb)
                    nc.vector.reduce_sum(out=sq[:, 4 * g: 4 * (g + 1)], in_=scr,
                                         axis=mybir.AxisListType.X)
                    xbs.append(xb)
                # one rsqrt per region: single ACT table load per function
                nc.scalar.activation(
                    out=rn, in_=sq,
                    func=mybir.ActivationFunctionType.Abs_reciprocal_sqrt)
                for g, xb in enumerate(xbs):
                    for t in range(4):
                        zrow = zrow_pool.tile([128, D], BF16)
                        nc.vector.tensor_scalar_mul(
                            out=zrow, in0=xb[:, t, :],
                            scalar1=rn[:, 4 * g + t: 4 * g + t + 1])
                        nc.gpsimd.dma_start(
                            out=dst_dram[(4 * g + t) * 128:(4 * g + t + 1) * 128, :],
                            in_=zrow)

            # ---- transposed reloads: [rows, 128 feat] -> [128, rows] ----
            def load_zT(dst, src_dram, nrows):
                half = (nrows // 1024) * 512 if nrows > 1024 else nrows
                for k in range(4):
                    for (r0, r1) in ((0, half), (half, nrows)):
                        if r0 == r1:
                            continue
                        nc.sync.dma_start_transpose(
                            out=dst[:, k, r0:r1],
                            in_=src_dram[r0:r1, k * 128:(k + 1) * 128],
                        )

            # ---- matmul block: lhsT cols [m0..m0+512) of zlocT vs 512 rhs cols ----
            def do_block(lhs_m0, rhs, rhs_n0, acc, acc_idx, extract, eacc=None, eidx=0):
                """One 512x512 sim block: 4 m-tiles x (4 k accum) matmuls + exp."""
                for m in range(4):
                    ps = psum_pool.tile([128, CHUNK], F32, tag="mm")
                    for k in range(4):
                        nc.tensor.matmul(
                            ps,
                            zlocT[:, k, lhs_m0 + m * 128: lhs_m0 + (m + 1) * 128],
                            rhs[:, k, rhs_n0: rhs_n0 + CHUNK],
                            start=(k == 0), stop=(k == 3),
                        )
                    if extract:
                        ex = exp_pool.tile([128, CHUNK], F32, tag="exp")
                    else:
                        ex = scr_pool.tile([128, CHUNK], F32, tag="expscr")
                    nc.scalar.activation(
                        out=ex, in_=ps, func=mybir.ActivationFunctionType.Exp,
                        scale=SCALE, accum_out=acc[:, acc_idx + m: acc_idx + m + 1],
                    )
                    if extract:
                        scr = scr_pool.tile([128, CHUNK], F32, tag="extscr")
                        nc.vector.tensor_mul(out=scr, in0=ex, in1=masks[:, m, :])
                        nc.vector.reduce_sum(
                            out=eacc[:, eidx + m: eidx + m + 1], in_=scr,
                            axis=mybir.AxisListType.X)

            # ------------- emission order (pipelining-friendly) --------------
            # All rsqrts before any Exp => exactly 2 ACT table loads total.
            norm_region(loc_d.ap(), zloc_d.ap(), ROWS_LOC, sq_loc, rn_loc)
            load_zT(zlocT, zloc_d.ap(), ROWS_LOC)
            norm_region(cols_d.ap()[:ROWS_A, :], za_d.ap(), ROWS_A, sq_a, rn_a)
            load_zT(zTA, za_d.ap(), ROWS_A)
            norm_region(cols_d.ap()[ROWS_A:, :], zb_d.ap(), ROWS_B, sq_b, rn_b)
            load_zT(zTB, zb_d.ap(), ROWS_B)

            # diag blocks: only depend on zlocT -> PE starts early
            do_block(0, zlocT, 0, acc_q, 0, True, acc_d, 0)       # diagA
            do_block(512, zlocT, 512, acc_q, 4, True, acc_d, 4)   # diagB

            # jobA: chunkA x col-chunks t=1..8 (n=3 -> t4 pos, n=7 -> t8 pos)
            for n in range(8):
                extract = n in (3, 7)
                eidx = 0 if n == 3 else 4
                do_block(0, zTA, n * CHUNK, acc_off, n * 4, extract, acc_np, eidx)

            # jobB: chunkB x col-chunks t=1..7 (n=3 -> t4 pos)
            for n in range(7):
                extract = n == 3
                do_block(512, zTB, n * CHUNK, acc_off, 32 + n * 4, extract,
                         acc_np, 8)

            # ---- final reduction: 4 categories -> [128,1] -> partition sum ----
            fin = persist.tile([128, 4], F32)
            for i, (acc, w) in enumerate(
                    [(acc_off, NT_OFF), (acc_q, NT_Q), (acc_d, NT_D), (acc_np, NT_NP)]):
                nc.vector.reduce_sum(out=fin[:, i:i + 1], in_=acc[:, :w],
                                     axis=mybir.AxisListType.X)
            psf = psum_pool.tile([128, CHUNK], F32, tag="mm")
            nc.tensor.matmul(psf[0:1, 0:4], ones, fin, start=True, stop=True)
            fout = persist.tile([1, 4], F32)
            nc.vector.tensor_copy(out=fout, in_=psf[0:1, 0:4])
            nc.gpsimd.dma_start(out=out_d.ap(), in_=fout)

    nc.compile()
    return nc, "out"


def _host_inputs(emb_i: np.ndarray, emb_j: np.ndarray):
    """Pure slicing/concat: build the 8 per-core input maps."""
    rows = np.ascontiguousarray(
        np.concatenate([emb_i, emb_j], axis=0), dtype=np.float32)

    masks = np.zeros((4, 128, D), dtype=np.float32)
    for s in range(4):
        for p in range(128):
            masks[s, p, 128 * s + p] = 1.0

    def cyc(start_row, nrows):
        idx = (np.arange(start_row, start_row + nrows)) % N
        return rows[idx]

    in_maps = []
    for c in range(NCORES):
        chunk_a, chunk_b = c, 15 - c
        loc = np.concatenate(
            [rows[chunk_a * CHUNK:(chunk_a + 1) * CHUNK],
             rows[chunk_b * CHUNK:(chunk_b + 1) * CHUNK]], axis=0)
        cols_a = cyc((chunk_a + 1) * CHUNK, ROWS_A)
        cols_b = cyc((chunk_b + 1) * CHUNK % N, ROWS_B)
        in_maps.append({
            "loc": np.ascontiguousarray(loc),
            "cols": np.ascontiguousarray(np.concatenate([cols_a, cols_b], axis=0)),
            "masks": masks,
        })
    return in_maps


def _combine(parts):
    """parts: list of 8 arrays [1,4] (S_off, Q, D, Np) -> scalar loss."""
    tot = np.sum(np.stack([p.astype(np.float64).ravel() for p in parts]), axis=0)
    s_off, q, d, npos = tot
    nom = 2.0 * npos
    den = 2.0 * s_off + q - d - nom
    loss = -np.log(nom / den) / N
    return np.float32(loss)


def kernel(emb_i: np.ndarray, emb_j: np.ndarray) -> np.ndarray:
    if "prog" not in _CACHED:
        _CACHED["prog"] = _build_program()
    nc, out_name = _CACHED["prog"]
    in_maps = _host_inputs(np.asarray(emb_i), np.asarray(emb_j))
    res = run_bass_kernel_spmd(nc, in_maps, list(range(NCORES)))
    parts = [res.results[c][out_name] for c in range(NCORES)]
    return np.array(_combine(parts), dtype=np.float32)


# revision 13
# speedup vs baseline: 1.7011x; 1.1149x over previous
"""Trainium2 Bass kernel for nn_BLLoss_66494683676972.

Contrastive (SimCLR-like) loss over rep = [normalize(emb_i); normalize(emb_j)]
(n=8192 rows, D=512):

    sim = rep @ rep.T
    nom = sum(exp(2*diag(sim, +-{B, 2B, 3B})))          (B=2048)
    den = sum_{i!=j} exp(2*sim) - nom
    loss = -log(nom/den) / 8192

Sharding: sim is symmetric, so only a cyclic half-band is computed.  Rows are
split into 16 chunks of 512; chunk R needs column-chunks R+1..R+7 (and R+8 for
R<=7) plus its diagonal block.  Core c owns chunks {c, 15-c} -> 17 blocks of
512x512 per core, perfectly balanced.  Per-core column data is rotated on the
host so the SPMD device program uses only static offsets.

The positive diagonals and the main diagonal are extracted from the computed
blocks with mask-reduce ops (t=4 blocks carry d=+2048 / d=+6144-mirror pairs,
t=8 blocks carry d=+4096 pairs).  Each core emits 4 partial sums; the host
combines them (the gather/unshard step) into the scalar loss.

Pipeline per core: cast-to-bf16 DMA loads -> batched square/reduce (DVE) ->
per-region rsqrt (one ACT table load) -> row scale (DVE) -> bf16 scratch in
DRAM -> xbar DMA-transpose reloads -> bf16 matmuls (PE, fp32 PSUM) -> fused
exp+row-sum (ACT) -> mask-extract diagonals (DVE) -> partition-sum (PE).
"""

import numpy as np

import concourse.bass as bass
import concourse.tile as tile
from concourse import bacc, mybir
from concourse.bass_utils import run_bass_kernel_spmd

B = 2048
N = 4 * B            # 8192 rows in rep
D = 512
NCORES = 8
CHUNK = 512          # row-chunk granularity (16 chunks)
TAU = 0.5
SCALE = 1.0 / TAU    # 2.0

ROWS_LOC = 2 * CHUNK          # 1024
ROWS_A = 8 * CHUNK            # 4096   col chunks +1..+8 of chunkA
ROWS_B = 7 * CHUNK            # 3584   col chunks +1..+7 of chunkB

F32 = mybir.dt.float32
BF16 = mybir.dt.bfloat16

_CACHED = {}


def _build_program():
    """Build (nc, out_name) for the SPMD program run on each of the 8 cores."""
    nc = bacc.Bacc("TRN2", target_bir_lowering=False, debug=False)

    loc_d = nc.declare_dram_parameter("loc", [ROWS_LOC, D], F32, isOutput=False)
    cols_d = nc.declare_dram_parameter("cols", [ROWS_A + ROWS_B, D], F32, isOutput=False)
    masks_d = nc.declare_dram_parameter("masks", [4, 128, D], F32, isOutput=False)
    out_d = nc.declare_dram_parameter("out", [1, 4], F32, isOutput=True)

    # bf16 normalized-row scratch, one region per source so the transposed
    # reloads only wait on their own region's stores.
    zloc_d = nc.dram_tensor("zloc_scratch", [ROWS_LOC, D], BF16)
    za_d = nc.dram_tensor("za_scratch", [ROWS_A, D], BF16)
    zb_d = nc.dram_tensor("zb_scratch", [ROWS_B, D], BF16)

    with tile.TileContext(nc) as tc:
        with (
            tc.tile_pool(name="persist", bufs=1) as persist,
            tc.tile_pool(name="xin", bufs=12) as xin_pool,
            tc.tile_pool(name="zrow", bufs=4) as zrow_pool,
            tc.tile_pool(name="scratch", bufs=2) as scr_pool,
            tc.tile_pool(name="expout", bufs=4) as exp_pool,
            tc.tile_pool(name="psum", bufs=8, space=bass.MemorySpace.PSUM) as psum_pool,
        ):
            # ---- persistent SBUF tensors ----
            masks = persist.tile([128, 4, D], BF16)
            nc.gpsimd.dma_start(out=masks, in_=masks_d.ap().rearrange("s p c -> p s c"))

            # zT layout: [128 partitions (feature-within-k-chunk), k-chunk, cols]
            zlocT = persist.tile([128, 4, ROWS_LOC], BF16)
            zTA = persist.tile([128, 4, ROWS_A], BF16)
            zTB = persist.tile([128, 4, ROWS_B], BF16)

            ones = persist.tile([128, 1], F32)
            nc.vector.memset(ones, 1.0)

            # per-region norm vectors (sq sums -> rnorm), one column per row-tile
            sq_loc = persist.tile([128, ROWS_LOC // 128], F32)
            sq_a = persist.tile([128, ROWS_A // 128], F32)
            sq_b = persist.tile([128, ROWS_B // 128], F32)
            rn_loc = persist.tile([128, ROWS_LOC // 128], F32)
            rn_a = persist.tile([128, ROWS_A // 128], F32)
            rn_b = persist.tile([128, ROWS_B // 128], F32)

            # accumulator columns: one fp32 scalar per [128,512] tile processed
            NT_OFF = 60   # 32 jobA + 28 jobB off-diag block tiles
            NT_Q = 8      # diagA + diagB block tiles
            NT_D = 8      # main-diag extractions (from diag blocks)
            NT_NP = 12    # positive extractions (t4A, t8A, t4B)
            acc_off = persist.tile([128, NT_OFF], F32)
            acc_q = persist.tile([128, NT_Q], F32)
            acc_d = persist.tile([128, NT_D], F32)
            acc_np = persist.tile([128, NT_NP], F32)

            # ---- phase 1: cast-load rows (4 tiles/load), square+reduce ----
            def load_region(src_ap, nrows):
                xbs = []
                for g in range(nrows // 512):
                    xb = xin_pool.tile([128, 4, D], BF16)
                    # bf16 cast during SWDGE DMA; rows 512g..512g+512
                    nc.gpsimd.dma_start(
                        out=xb,
                        in_=src_ap[512 * g: 512 * (g + 1), :].rearrange(
                            "(a p) d -> p a d", p=128))
                    xbs.append(xb)
                return xbs

            def norm_region(xbs, dst_dram, nrows, sq, rn):
                for g, xb in enumerate(xbs):
                    scr = scr_pool.tile([128, 4, D], BF16, tag="normscr")
                    nc.vector.tensor_mul(out=scr, in0=xb, in1=xb)
                    nc.vector.reduce_sum(out=sq[:, 4 * g: 4 * (g + 1)], in_=scr,
                                         axis=mybir.AxisListType.X)
                    # per-group rsqrt: tiny ACT op, frees xb quickly
                    nc.scalar.activation(
                        out=rn[:, 4 * g: 4 * (g + 1)],
                        in_=sq[:, 4 * g: 4 * (g + 1)],
                        func=mybir.ActivationFunctionType.Abs_reciprocal_sqrt)
                    zrow = zrow_pool.tile([128, 4, D], BF16)
                    for t in range(4):
                        nc.vector.tensor_scalar_mul(
                            out=zrow[:, t, :], in0=xb[:, t, :],
                            scalar1=rn[:, 4 * g + t: 4 * g + t + 1])
                    # one grouped store on the HWDGE (scalar) ring
                    nc.scalar.dma_start(
                        out=dst_dram[512 * g: 512 * (g + 1), :].rearrange(
                            "(a p) d -> p a d", p=128),
                        in_=zrow)

            # ---- transposed reloads: [rows, 128 feat] -> [128, rows] ----
            def load_zT(dst, src_dram, nrows):
                half = (nrows // 1024) * 512 if nrows > 1024 else nrows
                for k in range(4):
                    for (r0, r1) in ((0, half), (half, nrows)):
                        if r0 == r1:
                            continue
                        nc.sync.dma_start_transpose(
                            out=dst[:, k, r0:r1],
                            in_=src_dram[r0:r1, k * 128:(k + 1) * 128],
                        )

            # ---- matmul block: lhsT cols [m0..m0+512) of zlocT vs 512 rhs cols ----
            def do_block(lhs_m0, rhs, rhs_n0, acc, acc_idx, extract, eacc=None, eidx=0):
                """One 512x512 sim block: 4 m-tiles x (4 k accum) matmuls + exp."""
                for m in range(4):
                    ps = psum_pool.tile([128, CHUNK], F32, tag="mm")
                    for k in range(4):
                        nc.tensor.matmul(
                            ps,
                            zlocT[:, k, lhs_m0 + m * 128: lhs_m0 + (m + 1) * 128],
                            rhs[:, k, rhs_n0: rhs_n0 + CHUNK],
                            start=(k == 0), stop=(k == 3),
                        )
                    if extract:
                        ex = exp_pool.tile([128, CHUNK], BF16, tag="exp")
                    else:
                        ex = scr_pool.tile([128, CHUNK], BF16, tag="expscr")
                    nc.scalar.activation(
                        out=ex, in_=ps, func=mybir.ActivationFunctionType.Exp,
                        scale=SCALE, accum_out=acc[:, acc_idx + m: acc_idx + m + 1],
                    )
                    if extract:
                        scr = scr_pool.tile([128, CHUNK], BF16, tag="extscr")
                        nc.vector.tensor_mul(out=scr, in0=ex, in1=masks[:, m, :])
                        nc.vector.reduce_sum(
                            out=eacc[:, eidx + m: eidx + m + 1], in_=scr,
                            axis=mybir.AxisListType.X)

            # ------------- emission order (pipelining-friendly) --------------
            # All loads first: the gpsimd FIFO has no data-dependent waits, so
            # later regions' loads are never head-of-line blocked by stores.
            # All rsqrts precede any Exp => exactly 2 ACT table loads total.
            xbs_loc = load_region(loc_d.ap(), ROWS_LOC)
            xbs_a = load_region(cols_d.ap()[:ROWS_A, :], ROWS_A)
            xbs_b = load_region(cols_d.ap()[ROWS_A:, :], ROWS_B)

            norm_region(xbs_loc, zloc_d.ap(), ROWS_LOC, sq_loc, rn_loc)
            load_zT(zlocT, zloc_d.ap(), ROWS_LOC)
            norm_region(xbs_a, za_d.ap(), ROWS_A, sq_a, rn_a)
            load_zT(zTA, za_d.ap(), ROWS_A)
            norm_region(xbs_b, zb_d.ap(), ROWS_B, sq_b, rn_b)
            load_zT(zTB, zb_d.ap(), ROWS_B)

            # diag blocks: only depend on zlocT -> PE starts early
            do_block(0, zlocT, 0, acc_q, 0, True, acc_d, 0)       # diagA
            do_block(512, zlocT, 512, acc_q, 4, True, acc_d, 4)   # diagB

            # jobA: chunkA x col-chunks t=1..8 (n=3 -> t4 pos, n=7 -> t8 pos)
            for n in range(8):
                extract = n in (3, 7)
                eidx = 0 if n == 3 else 4
                do_block(0, zTA, n * CHUNK, acc_off, n * 4, extract, acc_np, eidx)

            # jobB: chunkB x col-chunks t=1..7 (n=3 -> t4 pos)
            for n in range(7):
                extract = n == 3
                do_block(512, zTB, n * CHUNK, acc_off, 32 + n * 4, extract,
                         acc_np, 8)

            # ---- final reduction: 4 categories -> [128,1] -> partition sum ----
            fin = persist.tile([128, 4], F32)
            for i, (acc, w) in enumerate(
                    [(acc_off, NT_OFF), (acc_q, NT_Q), (acc_d, NT_D), (acc_np, NT_NP)]):
                nc.vector.reduce_sum(out=fin[:, i:i + 1], in_=acc[:, :w],
                                     axis=mybir.AxisListType.X)
            psf = psum_pool.tile([128, CHUNK], F32, tag="mm")
            nc.tensor.matmul(psf[0:1, 0:4], ones, fin, start=True, stop=True)
            fout = persist.tile([1, 4], F32)
            nc.vector.tensor_copy(out=fout, in_=psf[0:1, 0:4])
            nc.gpsimd.dma_start(out=out_d.ap(), in_=fout)

    nc.compile()
    return nc, "out"


def _host_inputs(emb_i: np.ndarray, emb_j: np.ndarray):
    """Pure slicing/concat: build the 8 per-core input maps."""
    rows = np.ascontiguousarray(
        np.concatenate([emb_i, emb_j], axis=0), dtype=np.float32)

    masks = np.zeros((4, 128, D), dtype=np.float32)
    for s in range(4):
        for p in range(128):
            masks[s, p, 128 * s + p] = 1.0

    def cyc(start_row, nrows):
        idx = (np.arange(start_row, start_row + nrows)) % N
        return rows[idx]

    in_maps = []
    for c in range(NCORES):
        chunk_a, chunk_b = c, 15 - c
        loc = np.concatenate(
            [rows[chunk_a * CHUNK:(chunk_a + 1) * CHUNK],
             rows[chunk_b * CHUNK:(chunk_b + 1) * CHUNK]], axis=0)
        cols_a = cyc((chunk_a + 1) * CHUNK, ROWS_A)
        cols_b = cyc((chunk_b + 1) * CHUNK % N, ROWS_B)
        in_maps.append({
            "loc": np.ascontiguousarray(loc),
            "cols": np.ascontiguousarray(np.concatenate([cols_a, cols_b], axis=0)),
            "masks": masks,
        })
    return in_maps


def _combine(parts):
    """parts: list of 8 arrays [1,4] (S_off, Q, D, Np) -> scalar loss."""
    tot = np.sum(np.stack([p.astype(np.float64).ravel() for p in parts]), axis=0)
    s_off, q, d, npos = tot
    nom = 2.0 * npos
    den = 2.0 * s_off + q - d - nom
    loss = -np.log(nom / den) / N
    return np.float32(loss)


def kernel(emb_i: np.ndarray, emb_j: np.ndarray) -> np.ndarray:
    if "prog" not in _CACHED:
        _CACHED["prog"] = _build_program()
    nc, out_name = _CACHED["prog"]
    in_maps = _host_inputs(np.asarray(emb_i), np.asarray(emb_j))
    res = run_bass_kernel_spmd(nc, in_maps, list(range(NCORES)))
    parts = [res.results[c][out_name] for c in range(NCORES)]
    return np.array(_combine(parts), dtype=np.float32)


# revision 14
# speedup vs baseline: 1.8723x; 1.1006x over previous
"""Trainium2 Bass kernel for nn_BLLoss_66494683676972.

Contrastive (SimCLR-like) loss over rep = [normalize(emb_i); normalize(emb_j)]
(n=8192 rows, D=512):

    sim = rep @ rep.T
    nom = sum(exp(2*diag(sim, +-{B, 2B, 3B})))          (B=2048)
    den = sum_{i!=j} exp(2*sim) - nom
    loss = -log(nom/den) / 8192

Sharding: sim is symmetric, so only a cyclic half-band is computed.  Rows are
split into 16 chunks of 512; chunk R needs column-chunks R+1..R+7 (and R+8 for
R<=7) plus its diagonal block.  Core c owns chunks {c, 15-c} -> 17 blocks of
512x512 per core, perfectly balanced.  Per-core column data is rotated on the
host so the SPMD device program uses only static offsets.

The positive diagonals and the main diagonal are extracted from the computed
blocks with mask-reduce ops (t=4 blocks carry d=+2048 / d=+6144-mirror pairs,
t=8 blocks carry d=+4096 pairs).  Each core emits 4 partial sums; the host
combines them (the gather/unshard step) into the scalar loss.

Pipeline per core: cast-to-bf16 DMA loads -> batched square/reduce (DVE) ->
per-region rsqrt (one ACT table load) -> row scale (DVE) -> bf16 scratch in
DRAM -> xbar DMA-transpose reloads -> bf16 matmuls (PE, fp32 PSUM) -> fused
exp+row-sum (ACT) -> mask-extract diagonals (DVE) -> partition-sum (PE).
"""

import numpy as np

import concourse.bass as bass
import concourse.tile as tile
from concourse import bacc, mybir
from concourse.bass_utils import run_bass_kernel_spmd

B = 2048
N = 4 * B            # 8192 rows in rep
D = 512
NCORES = 8
CHUNK = 512          # row-chunk granularity (16 chunks)
TAU = 0.5
SCALE = 1.0 / TAU    # 2.0

ROWS_LOC = 2 * CHUNK          # 1024
ROWS_A = 8 * CHUNK            # 4096   col chunks +1..+8 of chunkA
ROWS_B = 7 * CHUNK            # 3584   col chunks +1..+7 of chunkB

F32 = mybir.dt.float32
BF16 = mybir.dt.bfloat16

_CACHED = {}


def _build_program():
    """Build (nc, out_name) for the SPMD program run on each of the 8 cores."""
    nc = bacc.Bacc("TRN2", target_bir_lowering=False, debug=False)

    loc_d = nc.declare_dram_parameter("loc", [ROWS_LOC, D], F32, isOutput=False)
    cols_d = nc.declare_dram_parameter("cols", [ROWS_A + ROWS_B, D], F32, isOutput=False)
    masks_d = nc.declare_dram_parameter("masks", [4, 128, D], F32, isOutput=False)
    out_d = nc.declare_dram_parameter("out", [1, 4], F32, isOutput=True)

    # bf16 normalized-row scratch, one region per source so the transposed
    # reloads only wait on their own region's stores.
    zloc_d = nc.dram_tensor("zloc_scratch", [ROWS_LOC, D], BF16)
    za_d = nc.dram_tensor("za_scratch", [ROWS_A, D], BF16)
    zb_d = nc.dram_tensor("zb_scratch", [ROWS_B, D], BF16)

    with tile.TileContext(nc) as tc:
        with (
            tc.tile_pool(name="persist", bufs=1) as persist,
            tc.tile_pool(name="xin", bufs=12) as xin_pool,
            tc.tile_pool(name="zrow", bufs=4) as zrow_pool,
            tc.tile_pool(name="scratch", bufs=2) as scr_pool,
            tc.tile_pool(name="expout", bufs=4) as exp_pool,
            tc.tile_pool(name="psum", bufs=8, space=bass.MemorySpace.PSUM) as psum_pool,
        ):
            # ---- persistent SBUF tensors ----
            masks = persist.tile([128, 4, D], BF16)
            nc.gpsimd.dma_start(out=masks, in_=masks_d.ap().rearrange("s p c -> p s c"))

            # zT layout: [128 partitions (feature-within-k-chunk), k-chunk, cols]
            zlocT = persist.tile([128, 4, ROWS_LOC], BF16)
            zTA = persist.tile([128, 4, ROWS_A], BF16)
            zTB = persist.tile([128, 4, ROWS_B], BF16)

            ones = persist.tile([128, 1], F32)
            nc.vector.memset(ones, 1.0)

            # per-region norm vectors (sq sums -> rnorm), one column per row-tile
            sq_loc = persist.tile([128, ROWS_LOC // 128], F32)
            sq_a = persist.tile([128, ROWS_A // 128], F32)
            sq_b = persist.tile([128, ROWS_B // 128], F32)
            rn_loc = persist.tile([128, ROWS_LOC // 128], F32)
            rn_a = persist.tile([128, ROWS_A // 128], F32)
            rn_b = persist.tile([128, ROWS_B // 128], F32)

            # accumulator columns: one fp32 scalar per [128,512] tile processed
            NT_OFF = 60   # 32 jobA + 28 jobB off-diag block tiles
            NT_Q = 8      # diagA + diagB block tiles
            NT_D = 8      # main-diag extractions (from diag blocks)
            NT_NP = 12    # positive extractions (t4A, t8A, t4B)
            acc_off = persist.tile([128, NT_OFF], F32)
            acc_q = persist.tile([128, NT_Q], F32)
            acc_d = persist.tile([128, NT_D], F32)
            acc_np = persist.tile([128, NT_NP], F32)

            # ---- phase 1: cast-load rows (4 tiles/load), square+reduce ----
            def load_region(src_ap, nrows):
                xbs = []
                for g in range(nrows // 512):
                    xb = xin_pool.tile([128, 4, D], BF16)
                    # bf16 cast during SWDGE DMA; rows 512g..512g+512
                    nc.gpsimd.dma_start(
                        out=xb,
                        in_=src_ap[512 * g: 512 * (g + 1), :].rearrange(
                            "(a p) d -> p a d", p=128))
                    xbs.append(xb)
                return xbs

            def norm_region(xbs, dst_dram, nrows, sq, rn):
                for g, xb in enumerate(xbs):
                    scr = scr_pool.tile([128, 4, D], BF16, tag="normscr")
                    nc.vector.tensor_mul(out=scr, in0=xb, in1=xb)
                    nc.vector.reduce_sum(out=sq[:, 4 * g: 4 * (g + 1)], in_=scr,
                                         axis=mybir.AxisListType.X)
                # region-level rsqrt: few big ACT ops -> no Exp/ars table thrash
                nc.scalar.activation(
                    out=rn, in_=sq,
                    func=mybir.ActivationFunctionType.Abs_reciprocal_sqrt)
                for g, xb in enumerate(xbs):
                    zrow = zrow_pool.tile([128, 4, D], BF16)
                    for t in range(4):
                        nc.vector.tensor_scalar_mul(
                            out=zrow[:, t, :], in0=xb[:, t, :],
                            scalar1=rn[:, 4 * g + t: 4 * g + t + 1])
                    # one grouped store on the HWDGE (scalar) ring
                    nc.scalar.dma_start(
                        out=dst_dram[512 * g: 512 * (g + 1), :].rearrange(
                            "(a p) d -> p a d", p=128),
                        in_=zrow)

            # ---- transposed reloads: [rows, 128 feat] -> [128, rows] ----
            def load_zT(dst, src_dram, nrows):
                half = (nrows // 1024) * 512 if nrows > 1024 else nrows
                for k in range(4):
                    for (r0, r1) in ((0, half), (half, nrows)):
                        if r0 == r1:
                            continue
                        nc.sync.dma_start_transpose(
                            out=dst[:, k, r0:r1],
                            in_=src_dram[r0:r1, k * 128:(k + 1) * 128],
                        )

            # ---- matmul block: lhsT cols [m0..m0+512) of zlocT vs 512 rhs cols ----
            def do_block(lhs_m0, rhs, rhs_n0, acc, acc_idx, extract, eacc=None, eidx=0):
                """One 512x512 sim block: 4 m-tiles x (4 k accum) matmuls + exp."""
                for m in range(4):
                    ps = psum_pool.tile([128, CHUNK], F32, tag="mm")
                    for k in range(4):
                        nc.tensor.matmul(
                            ps,
                            zlocT[:, k, lhs_m0 + m * 128: lhs_m0 + (m + 1) * 128],
                            rhs[:, k, rhs_n0: rhs_n0 + CHUNK],
                            start=(k == 0), stop=(k == 3),
                        )
                    if extract:
                        ex = exp_pool.tile([128, CHUNK], BF16, tag="exp")
                    else:
                        ex = scr_pool.tile([128, CHUNK], BF16, tag="expscr")
                    nc.scalar.activation(
                        out=ex, in_=ps, func=mybir.ActivationFunctionType.Exp,
                        scale=SCALE, accum_out=acc[:, acc_idx + m: acc_idx + m + 1],
                    )
                    if extract:
                        scr = scr_pool.tile([128, CHUNK], BF16, tag="extscr")
                        nc.vector.tensor_mul(out=scr, in0=ex, in1=masks[:, m, :])
                        nc.vector.reduce_sum(
                            out=eacc[:, eidx + m: eidx + m + 1], in_=scr,
                            axis=mybir.AxisListType.X)

            # ------------- emission order (pipelining-friendly) --------------
            # All loads first: the gpsimd FIFO has no data-dependent waits, so
            # later regions' loads are never head-of-line blocked by stores.
            # All rsqrts precede any Exp => exactly 2 ACT table loads total.
            xbs_loc = load_region(loc_d.ap(), ROWS_LOC)
            xbs_a = load_region(cols_d.ap()[:ROWS_A, :], ROWS_A)
            xbs_b = load_region(cols_d.ap()[ROWS_A:, :], ROWS_B)

            norm_region(xbs_loc, zloc_d.ap(), ROWS_LOC, sq_loc, rn_loc)
            load_zT(zlocT, zloc_d.ap(), ROWS_LOC)
            norm_region(xbs_a, za_d.ap(), ROWS_A, sq_a, rn_a)
            load_zT(zTA, za_d.ap(), ROWS_A)
            norm_region(xbs_b, zb_d.ap(), ROWS_B, sq_b, rn_b)
            load_zT(zTB, zb_d.ap(), ROWS_B)

            # diag blocks: only depend on zlocT -> PE starts early
            do_block(0, zlocT, 0, acc_q, 0, True, acc_d, 0)       # diagA
            do_block(512, zlocT, 512, acc_q, 4, True, acc_d, 4)   # diagB

            # jobA: chunkA x col-chunks t=1..8 (n=3 -> t4 pos, n=7 -> t8 pos)
            for n in range(8):
                extract = n in (3, 7)
                eidx = 0 if n == 3 else 4
                do_block(0, zTA, n * CHUNK, acc_off, n * 4, extract, acc_np, eidx)

            # jobB: chunkB x col-chunks t=1..7 (n=3 -> t4 pos)
            for n in range(7):
                extract = n == 3
                do_block(512, zTB, n * CHUNK, acc_off, 32 + n * 4, extract,
                         acc_np, 8)

            # ---- final reduction: 4 categories -> [128,1] -> partition sum ----
            fin = persist.tile([128, 4], F32)
            for i, (acc, w) in enumerate(
                    [(acc_off, NT_OFF), (acc_q, NT_Q), (acc_d, NT_D), (acc_np, NT_NP)]):
                nc.vector.reduce_sum(out=fin[:, i:i + 1], in_=acc[:, :w],
                                     axis=mybir.AxisListType.X)
            psf = psum_pool.tile([128, CHUNK], F32, tag="mm")
            nc.tensor.matmul(psf[0:1, 0:4], ones, fin, start=True, stop=True)
            fout = persist.tile([1, 4], F32)
            nc.vector.tensor_copy(out=fout, in_=psf[0:1, 0:4])
            nc.gpsimd.dma_start(out=out_d.ap(), in_=fout)

    nc.compile()
    return nc, "out"


def _host_inputs(emb_i: np.ndarray, emb_j: np.ndarray):
    """Pure slicing/concat: build the 8 per-core input maps."""
    rows = np.ascontiguousarray(
        np.concatenate([emb_i, emb_j], axis=0), dtype=np.float32)

    masks = np.zeros((4, 128, D), dtype=np.float32)
    for s in range(4):
        for p in range(128):
            masks[s, p, 128 * s + p] = 1.0

    def cyc(start_row, nrows):
        idx = (np.arange(start_row, start_row + nrows)) % N
        return rows[idx]

    in_maps = []
    for c in range(NCORES):
        chunk_a, chunk_b = c, 15 - c
        loc = np.concatenate(
            [rows[chunk_a * CHUNK:(chunk_a + 1) * CHUNK],
             rows[chunk_b * CHUNK:(chunk_b + 1) * CHUNK]], axis=0)
        cols_a = cyc((chunk_a + 1) * CHUNK, ROWS_A)
        cols_b = cyc((chunk_b + 1) * CHUNK % N, ROWS_B)
        in_maps.append({
            "loc": np.ascontiguousarray(loc),
            "cols": np.ascontiguousarray(np.concatenate([cols_a, cols_b], axis=0)),
            "masks": masks,
        })
    return in_maps


def _combine(parts):
    """parts: list of 8 arrays [1,4] (S_off, Q, D, Np) -> scalar loss."""
    tot = np.sum(np.stack([p.astype(np.float64).ravel() for p in parts]), axis=0)
    s_off, q, d, npos = tot
    nom = 2.0 * npos
    den = 2.0 * s_off + q - d - nom
    loss = -np.log(nom / den) / N
    return np.float32(loss)


def kernel(emb_i: np.ndarray, emb_j: np.ndarray) -> np.ndarray:
    if "prog" not in _CACHED:
        _CACHED["prog"] = _build_program()
    nc, out_name = _CACHED["prog"]
    in_maps = _host_inputs(np.asarray(emb_i), np.asarray(emb_j))
    res = run_bass_kernel_spmd(nc, in_maps, list(range(NCORES)))
    parts = [res.results[c][out_name] for c in range(NCORES)]
    return np.array(_combine(parts), dtype=np.float32)
